# revision 36
# baseline (speedup 1.0000x reference)
"""Distributed 2-layer GCN (PyG GCNConv semantics) on 8 Trainium2 NeuronCores.

Strategy (graph/data parallel, per sharding hint):
- Nodes sharded by contiguous range across 8 cores; edges sharded by dst owner.
- Node tables (g1 = dinv*(x@W1), g2 = dinv^2*(relu(agg+sqrtdeg*b1)@W2)) use one
  core-major padded row layout, so both layers share a single edge-index
  structure.
- Layer tables are built SHARDED (each core transforms its own nodes) and
  exchanged with AllGather into a Shared DRAM table per core.
- Edge aggregation = sorted-by-dst gather (dma_gather on 4 SWDGE queues,
  1024-index single-packet calls, merged cells over CHUNK dst-groups per
  window) + one-hot selection matmul in PSUM.
- Layer-1 accumulates TRANSPOSED (hid on partitions) so relu applies in place
  and the layer-2 transform needs no PE transpose; b1 enters as a rank-1
  matmul against sqrt(deg).
"""
import numpy as np
import ml_dtypes

import concourse.bass as bass
import concourse.mybir as mybir
import concourse.tile as tile
from concourse import bacc
from concourse.bass_utils import run_bass_kernel_spmd

F32 = mybir.dt.float32
BF16 = mybir.dt.bfloat16
I16 = mybir.dt.int16

P = 128
NCORES = 8
CHUNK = 2          # dst-groups merged per gather cell

# problem sizes (hardcoded per spec)
N_NODES = 100000
NFEAT = 512
NHID = 256
NCLS = 40

_prog_cache = {}

_PHASES = ["B", "G", "C", "D", "E"]


def _chunks(G):
    return [list(range(c, min(c + CHUNK, G))) for c in range(0, G, CHUNK)]


# --------------------------------------------------------------------------
# program builder
# --------------------------------------------------------------------------
def build_program(cfg):
    import os
    max_phase = _PHASES.index(os.environ.get("GCN_MAX_PHASE", "E"))
    nfeat, nhid, ncls = cfg["nfeat"], cfg["nhid"], cfg["ncls"]
    ntab, nwin, wrow = cfg["ntab"], cfg["nwin"], cfg["wrow"]
    nloc = cfg["nloc"]
    S = cfg["S"]
    G = nloc // P
    KC = nfeat // P      # k-chunks for transform
    HC = nhid // P       # k-chunks for layer-2 transform
    NHPAD = P            # padded g2 row length (bf16 -> 256B)

    chunks = _chunks(G)
    # per-g block counts and index-column counts
    Bg = [sum(S[g][q] // P for q in range(nwin)) for g in range(G)]
    Cg = [sum(S[g][q] // 16 for q in range(nwin)) for g in range(G)]

    nc = bacc.Bacc(num_swdge_queues=4)
    qrr = [0]  # round-robin SWDGE queue assignment for gathers

    def next_q():
        qrr[0] = (qrr[0] + 1) % 4
        return qrr[0]

    # ---- external inputs ----
    A_in = nc.dram_tensor("A", [G, P, KC * P], BF16, kind="ExternalInput")
    W1_in = nc.dram_tensor("W1c", [P, KC * nhid], BF16, kind="ExternalInput")
    W2_in = nc.dram_tensor("W2c", [P, HC * ncls], BF16, kind="ExternalInput")
    b1r_in = nc.dram_tensor("b1r", [1, nhid], BF16, kind="ExternalInput")
    b2_in = nc.dram_tensor("b2b", [P, ncls], F32, kind="ExternalInput")
    iota_in = nc.dram_tensor("iota", [P, P], F32, kind="ExternalInput")
    degl_in = nc.dram_tensor("degl", [P, G], F32, kind="ExternalInput")
    sdeg_in = nc.dram_tensor("sdegr", [1, nloc], BF16, kind="ExternalInput")
    idx_in = nc.dram_tensor("idx", [P, sum(Cg)], I16, kind="ExternalInput")
    da_in = nc.dram_tensor("da", [P, sum(Bg)], F32, kind="ExternalInput")

    out_ext = nc.dram_tensor("out", [nloc, ncls], F32, kind="ExternalOutput")

    # ---- internal DRAM ----
    dbg = os.environ.get("GCN_DEBUG") == "1"
    g1_loc = nc.dram_tensor("g1_loc", [nloc, nhid], BF16)
    g1_tab = nc.dram_tensor("g1_tab", [ntab, nhid], BF16, addr_space="Shared")
    g2_loc = nc.dram_tensor("g2_loc", [nloc, NHPAD], BF16)
    if dbg:
        g1_dbg = nc.dram_tensor("g1_dbg", [nloc, nhid], BF16,
                                kind="ExternalOutput")
        g2_dbg = nc.dram_tensor("g2_dbg", [nloc, NHPAD], BF16,
                                kind="ExternalOutput")
    g2_tab = nc.dram_tensor("g2_tab", [ntab, NHPAD], BF16, addr_space="Shared")

    AF = mybir.ActivationFunctionType

    with tile.TileContext(nc) as tc:
        with (
            tc.tile_pool(name="const", bufs=1) as cpool,
            tc.tile_pool(name="xf", bufs=3) as xfpool,
            tc.tile_pool(name="meta", bufs=3) as mpool,
            tc.tile_pool(name="gat", bufs=2) as gpool,
            tc.tile_pool(name="sel", bufs=2) as spool,
            tc.tile_pool(name="epi", bufs=3) as epool,
            tc.tile_pool(name="psA", bufs=2, space="PSUM") as psA,
            tc.tile_pool(name="psB", bufs=2, space="PSUM") as psB,
        ):
            # ---- constants ----
            w1_t = cpool.tile([P, KC * nhid], BF16)
            nc.sync.dma_start(out=w1_t[:], in_=W1_in[:, :])
            w2_t = cpool.tile([P, HC * ncls], BF16)
            nc.sync.dma_start(out=w2_t[:], in_=W2_in[:, :])
            b1r_t = cpool.tile([1, nhid], BF16)
            nc.sync.dma_start(out=b1r_t[:], in_=b1r_in[:, :])
            b2_t = cpool.tile([P, ncls], F32)
            nc.sync.dma_start(out=b2_t[:], in_=b2_in[:, :])
            iota_t = cpool.tile([P, P], F32)
            nc.sync.dma_start(out=iota_t[:], in_=iota_in[:, :])
            sdeg_t = cpool.tile([1, nloc], BF16)
            nc.sync.dma_start(out=sdeg_t[:], in_=sdeg_in[:, :])

            degl_t = cpool.tile([P, G], F32)
            nc.sync.dma_start(out=degl_t[:], in_=degl_in[:, :])
            dinvl_t = cpool.tile([P, G], F32)
            nc.vector.reciprocal(out=dinvl_t[:], in_=degl_t[:])
            nc.scalar.activation(out=dinvl_t[:], in_=dinvl_t[:], func=AF.Sqrt)
            dinv2_t = cpool.tile([P, G], F32)
            nc.vector.reciprocal(out=dinv2_t[:], in_=degl_t[:])

            # ---- phase B: sharded transform -> g1_loc ----
            for t in range(G):
                a_t = xfpool.tile([P, KC * P], BF16, tag="a")
                nc.sync.dma_start(out=a_t[:], in_=A_in[t])
                ps = psB.tile([P, nhid], F32, tag="g2")
                for c in range(KC):
                    nc.tensor.matmul(
                        out=ps[:], lhsT=a_t[:, c * P:(c + 1) * P],
                        rhs=w1_t[:, c * nhid:(c + 1) * nhid],
                        start=(c == 0), stop=(c == KC - 1),
                    )
                gsb = xfpool.tile([P, nhid], BF16, tag="gout")
                nc.scalar.activation(out=gsb[:], in_=ps[:], func=AF.Copy,
                                     scale=dinvl_t[:, t:t + 1])
                nc.sync.dma_start(out=g1_loc[t * P:(t + 1) * P, :], in_=gsb[:])
                if dbg:
                    nc.sync.dma_start(out=g1_dbg[t * P:(t + 1) * P, :],
                                      in_=gsb[:])

            # ---- phase G: exchange g1 ----
            if max_phase >= 1:
                nc.gpsimd.collective_compute(
                    "AllGather", mybir.AluOpType.bypass,
                    replica_groups=[list(range(NCORES))],
                    ins=[g1_loc[:, :]], outs=[g1_tab[:, :]],
                )

            # ---- shared chunk machinery for phases C and E ----
            # slot order within a chunk: [q][g in chunk][slots (128-padded)]
            def agg_phase(tab, width, rwidth, emit_accs, emit_epilogue):
                """tab: gather source; width: gathered row elems; rwidth:
                rhs width used by matmuls; emit_accs(chunk)->accs;
                emit_epilogue(g, acc) emits the per-group epilogue."""
                co_base = 0
                bo_base = 0
                pend = []
                for chunk in chunks:
                    ccols = sum(Cg[g] for g in chunk)
                    bcols = sum(Bg[g] for g in chunk)
                    idx_t = mpool.tile([P, ccols], I16, tag="idx")
                    nc.sync.dma_start(out=idx_t[:],
                                      in_=idx_in[:, co_base:co_base + ccols])
                    da_t = mpool.tile([P, bcols], F32, tag="da")
                    nc.sync.dma_start(out=da_t[:],
                                      in_=da_in[:, bo_base:bo_base + bcols])

                    gat_t = gpool.tile([P, bcols, width], BF16, tag="gat")
                    co = 0
                    bo = 0
                    # block layout bookkeeping: (q, g) -> block offset
                    blk_of = {}
                    for q in range(nwin):
                        cell = sum(S[g][q] for g in chunk)
                        b0 = bo
                        for g in chunk:
                            blk_of[(q, g)] = bo
                            bo += S[g][q] // P
                        for s0 in range(0, cell, 1024):
                            ss = min(1024, cell - s0)
                            nc.gpsimd.dma_gather(
                                gat_t[:, b0 + s0 // P:b0 + (s0 + ss) // P, :],
                                tab[q * wrow:(q + 1) * wrow, :],
                                idx_t[:, co + s0 // 16:co + (s0 + ss) // 16],
                                ss, ss, width, queue_num=next_q(),
                            )
                        co += cell // 16

                    sel = spool.tile([P, bcols, P], BF16, tag="sel")
                    nc.vector.tensor_tensor(
                        out=sel[:],
                        in0=da_t[:, 0:bcols].unsqueeze(2)
                            .to_broadcast([P, bcols, P]),
                        in1=iota_t[:].unsqueeze(1).to_broadcast([P, bcols, P]),
                        op=mybir.AluOpType.is_equal)

                    accs = emit_accs(chunk)
                    # per-g first/last block flags across windows
                    nz = {g: [q for q in range(nwin) if S[g][q] > 0]
                          for g in chunk}
                    for q in range(nwin):
                        for j, g in enumerate(chunk):
                            nb = S[g][q] // P
                            if nb == 0:
                                continue
                            first_q = nz[g][0]
                            last_q = nz[g][-1]
                            for b in range(nb):
                                bi = blk_of[(q, g)] + b
                                is_first = (q == first_q and b == 0)
                                is_last = (q == last_q and b == nb - 1)
                                emit_accs.mm(accs[j], gat_t, sel, bi,
                                             is_first, is_last)
                    # software-pipelined epilogues (previous chunk)
                    for (gp, accp) in pend:
                        emit_epilogue(gp, accp)
                    pend = [(g, accs[j]) for j, g in enumerate(chunk)]

                    co_base += ccols
                    bo_base += bcols
                for (gp, accp) in pend:
                    emit_epilogue(gp, accp)

            # ---- phase C: L1 aggregation + fused layer-2 transform ----
            # accT[hid, d] accumulated transposed; halves of one PSUM tile
            # hold the two hid chunks. out1 = dinv*relu(agg + sqrtdeg*b1);
            # g2 = dinv^2 * (relu(...) @ W2).
            if max_phase >= 2:
                def c_accs(chunk):
                    accs = []
                    for j, g in enumerate(chunk):
                        acc = [psA.tile([P, P], F32, tag=f"acc{j}h{h}",
                                        name=f"acch{h}") for h in range(HC)]
                        for h in range(HC):
                            nc.tensor.matmul(
                                out=acc[h][:],
                                lhsT=b1r_t[0:1, h * P:(h + 1) * P],
                                rhs=sdeg_t[0:1, g * P:(g + 1) * P],
                                start=True, stop=False)
                        accs.append(acc)
                    return accs

                def c_mm(acc, gat_t, sel, bi, is_first, is_last):
                    for h in range(HC):
                        nc.tensor.matmul(
                            out=acc[h][:],
                            lhsT=gat_t[:, bi, h * P:(h + 1) * P],
                            rhs=sel[:, bi, :],
                            start=False, stop=is_last)
                c_accs.mm = c_mm

                def c_epilogue(g, acc):
                    us = []
                    for h in range(HC):
                        u = epool.tile([P, P], BF16, tag=f"u{h}",
                                       name=f"u{h}")
                        nc.scalar.activation(out=u[:], in_=acc[h][:],
                                             func=AF.Relu)
                        us.append(u)
                    g2ps = psB.tile([P, ncls], F32, tag="g2")
                    for h in range(HC):
                        nc.tensor.matmul(out=g2ps[:], lhsT=us[h][:],
                                         rhs=w2_t[:, h * ncls:(h + 1) * ncls],
                                         start=(h == 0), stop=(h == HC - 1))
                    g2sb = epool.tile([P, NHPAD], BF16, tag="g2sb")
                    nc.scalar.activation(out=g2sb[:, 0:ncls], in_=g2ps[:],
                                         func=AF.Copy,
                                         scale=dinv2_t[:, g:g + 1])
                    nc.sync.dma_start(out=g2_loc[g * P:(g + 1) * P, :],
                                      in_=g2sb[:])
                    if dbg:
                        nc.sync.dma_start(out=g2_dbg[g * P:(g + 1) * P, :],
                                          in_=g2sb[:])

                agg_phase(g1_tab, nhid, nhid, c_accs, c_epilogue)

            # ---- phase D: exchange g2 ----
            if max_phase >= 3:
                nc.gpsimd.collective_compute(
                    "AllGather", mybir.AluOpType.bypass,
                    replica_groups=[list(range(NCORES))],
                    ins=[g2_loc[:, :]], outs=[g2_tab[:, :]],
                )

            # ---- phase E: L2 aggregation + log_softmax ----
            if max_phase >= 4:
                def e_accs(chunk):
                    accs = []
                    for j, g in enumerate(chunk):
                        acc = psA.tile([P, ncls], F32, tag=f"acc{j}",
                                       name=f"acc{j}")
                        accs.append(acc)
                    return accs

                def e_mm(acc, gat_t, sel, bi, is_first, is_last):
                    nc.tensor.matmul(out=acc[:], lhsT=sel[:, bi, :],
                                     rhs=gat_t[:, bi, 0:ncls],
                                     start=is_first, stop=is_last)
                e_accs.mm = e_mm

                def e_epilogue(g, acc):
                    t1 = epool.tile([P, ncls], F32, tag="e1")
                    nc.scalar.activation(out=t1[:], in_=acc[:], func=AF.Copy,
                                         scale=dinvl_t[:, g:g + 1])
                    o2 = epool.tile([P, ncls], F32, tag="e2")
                    nc.vector.tensor_tensor(out=o2[:], in0=t1[:], in1=b2_t[:],
                                            op=mybir.AluOpType.add)
                    negm = epool.tile([P, 1], F32, tag="negm")
                    nc.vector.tensor_reduce(out=negm[:], in_=o2[:],
                                            op=mybir.AluOpType.max,
                                            axis=mybir.AxisListType.X,
                                            negate=True)
                    e_t = epool.tile([P, ncls], F32, tag="escr")
                    s_t = epool.tile([P, 1], F32, tag="ssum")
                    nc.scalar.activation(out=e_t[:], in_=o2[:], func=AF.Exp,
                                         bias=negm[:, 0:1],
                                         accum_out=s_t[:, 0:1])
                    l_t = epool.tile([P, 1], F32, tag="lsum")
                    nc.scalar.activation(out=l_t[:], in_=s_t[:], func=AF.Ln)
                    mpl = epool.tile([P, 1], F32, tag="mpl")
                    nc.vector.tensor_tensor(out=mpl[:], in0=l_t[:], in1=negm[:],
                                            op=mybir.AluOpType.subtract)
                    fin = epool.tile([P, ncls], F32, tag="fin")
                    nc.vector.tensor_scalar(out=fin[:], in0=o2[:],
                                            scalar1=mpl[:, 0:1],
                                            scalar2=None,
                                            op0=mybir.AluOpType.subtract)
                    nc.sync.dma_start(out=out_ext[g * P:(g + 1) * P, :],
                                      in_=fin[:])

                agg_phase(g2_tab, NHPAD, ncls, e_accs, e_epilogue)

    nc.compile()
    return nc


# --------------------------------------------------------------------------
# host-side data prep
# --------------------------------------------------------------------------
def _wrap_idx_cols(vals, S):
    """vals: int array of S slot indices -> [128, S//16] int16 (16-wrapped, x8)"""
    w = vals.reshape(S // 16, 16).T.astype(np.int16)  # [16, S/16]
    return np.tile(w, (8, 1))


def prepare(x, edge_index, W1, b1, W2, b2):
    n, nfeat = x.shape
    nhid = W1.shape[1]
    ncls = W2.shape[1]
    assert n % NCORES == 0
    nown = n // NCORES                       # real nodes per core
    nloc = -(-nown // P) * P                 # padded local nodes
    ntab = NCORES * nloc                     # core-major table rows
    nwin = 4
    assert ntab % nwin == 0
    wrow = ntab // nwin
    assert wrow < 32768
    G = nloc // P

    src = np.asarray(edge_index[0], dtype=np.int64)
    dst = np.asarray(edge_index[1], dtype=np.int64)

    deg = np.bincount(dst, minlength=n).astype(np.float32) + 1.0

    # append self loops, sort by dst (stable keeps determinism)
    loops = np.arange(n, dtype=np.int64)
    src_all = np.concatenate([src, loops])
    dst_all = np.concatenate([dst, loops])
    order = np.argsort(dst_all, kind="stable")
    ssrc = src_all[order]
    sdst = dst_all[order]

    # src row in the core-major padded table (shared by both layers)
    core_of = ssrc // nown
    row2 = core_of * nloc + (ssrc - core_of * nown)
    we = row2 // wrow
    ie = (row2 - we * wrow).astype(np.int64)

    # per-core edge ranges (dst owner)
    cuts = np.searchsorted(sdst, np.arange(NCORES + 1) * nown)

    # first pass: per (core, g, q) counts
    cnt = np.zeros((NCORES, G, nwin), np.int64)
    per_core = []
    for k in range(NCORES):
        e0, e1 = cuts[k], cuts[k + 1]
        dl = (sdst[e0:e1] - k * nown).astype(np.int64)
        gid = dl // P
        gcuts = np.searchsorted(gid, np.arange(G + 1))
        per_core.append((e0, e1, dl, gcuts))
        for g in range(G):
            a, b = gcuts[g], gcuts[g + 1]
            cnt[k, g] = np.bincount(we[e0 + a:e0 + b], minlength=nwin)

    m = cnt.max(axis=0)                      # [G, nwin]
    S = (-(-m // P) * P).astype(np.int64)    # pad to 128, 0 stays 0

    chunks = _chunks(G)

    # second pass: build idx/dstadj arrays per core, chunk-major slot order:
    # [chunk][q][g in chunk][slots padded to S[g][q]]
    def build_layer(k):
        e0, e1, dl, gcuts = per_core[k]
        idx_cols = []
        da_cols = []
        for chunk in chunks:
            for q in range(nwin):
                for g in chunk:
                    a, b = gcuts[g], gcuts[g + 1]
                    wv = we[e0 + a:e0 + b]
                    iv = ie[e0 + a:e0 + b]
                    dv = dl[a:b] - g * P
                    S_gq = int(S[g, q])
                    if S_gq == 0:
                        continue
                    msk = wv == q
                    cntq = int(msk.sum())
                    vals = np.zeros(S_gq, np.int64)
                    vals[:cntq] = iv[msk]
                    dd = np.full(S_gq, -1e9, np.float32)
                    dd[:cntq] = dv[msk].astype(np.float32)
                    idx_cols.append(_wrap_idx_cols(vals, S_gq))
                    da_cols.append(dd.reshape(S_gq // P, P).T)
        return (np.concatenate(idx_cols, axis=1),
                np.ascontiguousarray(np.concatenate(da_cols, axis=1)))

    KC = nfeat // P
    HC = nhid // P

    W1c = (np.asarray(W1, np.float32).reshape(KC, P, nhid).transpose(1, 0, 2)
           .reshape(P, KC * nhid).astype(ml_dtypes.bfloat16))
    W2c = (np.asarray(W2, np.float32).reshape(HC, P, ncls).transpose(1, 0, 2)
           .reshape(P, HC * ncls).astype(ml_dtypes.bfloat16))
    b1r = np.asarray(b1, np.float32).reshape(1, nhid).astype(ml_dtypes.bfloat16)
    b2b = np.tile(np.asarray(b2, np.float32), (P, 1))
    iota = np.broadcast_to(np.arange(P, dtype=np.float32), (P, P)).copy()

    in_maps = []
    for k in range(NCORES):
        # per-core transform input: A[t][p, c*P+j] = x[k*nown + t*P+j, c*P+p]
        xk = np.zeros((nloc, nfeat), np.float32)
        xk[:nown] = x[k * nown:(k + 1) * nown]
        A = (xk.T.reshape(KC, P, G, P).transpose(2, 1, 0, 3)
             .reshape(G, P, KC * P).astype(ml_dtypes.bfloat16))

        dloc = np.ones(nloc, np.float32)
        dloc[:nown] = deg[k * nown:(k + 1) * nown]
        degl = dloc.reshape(G, P).T.copy()
        sdegr = np.sqrt(dloc).reshape(1, nloc).astype(ml_dtypes.bfloat16)
        idx, da = build_layer(k)
        in_maps.append({
            "A": A, "W1c": W1c, "W2c": W2c, "b1r": b1r, "b2b": b2b,
            "iota": iota, "degl": degl, "sdegr": sdegr,
            "idx": idx, "da": da,
        })

    cfg = {
        "nfeat": nfeat, "nhid": nhid, "ncls": ncls,
        "ntab": ntab, "nwin": nwin, "wrow": wrow, "nloc": nloc,
        "S": S.tolist(),
    }
    return cfg, in_maps, nown


def _run(x, edge_index, W1, b1, W2, b2, trace=False):
    cfg, in_maps, nown = prepare(x, edge_index, W1, b1, W2, b2)
    key = repr(sorted(cfg.items()))
    nc = _prog_cache.get(key)
    if nc is None:
        nc = build_program(cfg)
        _prog_cache[key] = nc
    res = run_bass_kernel_spmd(nc, in_maps, core_ids=list(range(NCORES)),
                               trace=trace)
    n = x.shape[0]
    ncls = W2.shape[1]
    out = np.empty((n, ncls), np.float32)
    for k in range(NCORES):
        out[k * nown:(k + 1) * nown] = res.results[k]["out"][:nown]
    return out, res


def kernel(x, edge_index, W1, b1, W2, b2):
    out, _ = _run(np.asarray(x), np.asarray(edge_index),
                  np.asarray(W1), np.asarray(b1), np.asarray(W2), np.asarray(b2))
    return out


# --------------------------------------------------------------------------
# timing harness (test.py only): stage inputs once, time repeated executions
# --------------------------------------------------------------------------
def build_timed_runner(nc, in_maps):
    """Mirror run_bass_via_pjrt's multi-core path, but keep inputs staged on
    device and return a callable that executes once and blocks."""
    import jax
    from jax.sharding import Mesh, PartitionSpec
    from jax.experimental.shard_map import shard_map
    from concourse import bass2jax
    from concourse.bass2jax import (_bass_exec_p, partition_id_tensor,
                                    fast_dispatch_compile)

    bass2jax.install_neuronx_cc_hook()
    n_cores = len(in_maps)

    partition_name = nc.partition_id_tensor.name if nc.partition_id_tensor else None
    in_names, out_names, out_avals, zero_outs = [], [], [], []
    for alloc in nc.m.functions[0].allocations:
        if not isinstance(alloc, mybir.MemoryLocationSet):
            continue
        name = alloc.memorylocations[0].name
        if alloc.kind == "ExternalInput":
            if name != partition_name:
                in_names.append(name)
        elif alloc.kind == "ExternalOutput":
            out_names.append(name)
            shape = tuple(alloc.tensor_shape)
            dtype = mybir.dt.np(alloc.dtype)
            out_avals.append(jax.core.ShapedArray(shape, dtype))
            zero_outs.append(np.zeros(shape, dtype))
    n_params = len(in_names)
    all_in_names = in_names + out_names + ([partition_name] if partition_name else [])

    def _body(*args):
        operands = list(args)
        if partition_name is not None:
            operands.append(partition_id_tensor())
        return tuple(_bass_exec_p.bind(
            *operands, out_avals=tuple(out_avals), in_names=tuple(all_in_names),
            out_names=tuple(out_names), lowering_input_output_aliases=(),
            sim_require_finite=True, sim_require_nnan=True, nc=nc))

    devices = jax.devices()[:n_cores]
    mesh = Mesh(np.asarray(devices), ("core",))
    n_outs = len(out_names)

    import time
    t0 = time.time()
    abstract = [jax.ShapeDtypeStruct(
        (n_cores * np.asarray(in_maps[0][nm]).shape[0],
         *np.asarray(in_maps[0][nm]).shape[1:]),
        np.asarray(in_maps[0][nm]).dtype) for nm in in_names]
    abstract += [jax.ShapeDtypeStruct((n_cores * z.shape[0], *z.shape[1:]), z.dtype)
                 for z in zero_outs]

    # No donation: the NEFF binds outputs to fresh result buffers and never
    # reads the zero "output" params, so one staged device-resident zeros
    # array can be reused every call (avoids a 16MB host->device upload
    # through the axon tunnel per execution). fast_dispatch_compile drops
    # the bass effect so dispatch takes the C++ fast path.
    def _compile():
        return jax.jit(
            shard_map(_body, mesh=mesh,
                      in_specs=(PartitionSpec("core"),) * (n_params + n_outs),
                      out_specs=(PartitionSpec("core"),) * n_outs,
                      check_rep=False),
            keep_unused=True).lower(*abstract).compile()

    sharded = fast_dispatch_compile(_compile)
    print(f"[runner] jit+neff compile: {time.time() - t0:.1f}s", flush=True)

    from jax.sharding import NamedSharding
    shard = NamedSharding(mesh, PartitionSpec("core"))
    staged = []
    for i, name in enumerate(in_names):
        cat = np.concatenate([np.asarray(m[name]) for m in in_maps], axis=0)
        staged.append(jax.device_put(cat, shard))
    jax.block_until_ready(staged)
    print(f"[runner] inputs staged: {time.time() - t0:.1f}s", flush=True)

    staged_zeros = []
    for z in zero_outs:
        zz = np.zeros((n_cores * z.shape[0], *z.shape[1:]), z.dtype)
        staged_zeros.append(jax.device_put(zz, shard))
    jax.block_until_ready(staged_zeros)

    def submit():
        return sharded(*staged, *staged_zeros)

    def run_once():
        out = submit()
        jax.block_until_ready(out)
        return out

    run_once.submit = submit
    return run_once, out_names, out_avals


# revision 37
# speedup vs baseline: 1.0172x; 1.0172x over previous
"""Distributed 2-layer GCN (PyG GCNConv semantics) on 8 Trainium2 NeuronCores.

Strategy (graph/data parallel, per sharding hint):
- Nodes sharded by contiguous range across 8 cores; edges sharded by dst owner.
- Node tables (g1 = dinv*(x@W1), g2 = dinv^2*(relu(agg+sqrtdeg*b1)@W2)) use one
  core-major padded row layout, so both layers share a single edge-index
  structure.
- Layer tables are built SHARDED (each core transforms its own nodes) and
  exchanged with AllGather into a Shared DRAM table per core.
- Edge aggregation = sorted-by-dst gather (dma_gather on 4 SWDGE queues,
  1024-index single-packet calls, merged cells over CHUNK dst-groups per
  window) + one-hot selection matmul in PSUM.
- Layer-1 accumulates TRANSPOSED (hid on partitions) so relu applies in place
  and the layer-2 transform needs no PE transpose; b1 enters as a rank-1
  matmul against sqrt(deg).
"""
import numpy as np
import ml_dtypes

import concourse.bass as bass
import concourse.mybir as mybir
import concourse.tile as tile
from concourse import bacc
from concourse.bass_utils import run_bass_kernel_spmd

F32 = mybir.dt.float32
BF16 = mybir.dt.bfloat16
I16 = mybir.dt.int16

P = 128
NCORES = 8
CHUNK = 2          # dst-groups merged per gather cell

# problem sizes (hardcoded per spec)
N_NODES = 100000
NFEAT = 512
NHID = 256
NCLS = 40

_prog_cache = {}

_PHASES = ["B", "G", "C", "D", "E"]


def _chunks(G):
    return [list(range(c, min(c + CHUNK, G))) for c in range(0, G, CHUNK)]


# --------------------------------------------------------------------------
# program builder
# --------------------------------------------------------------------------
def build_program(cfg):
    import os
    max_phase = _PHASES.index(os.environ.get("GCN_MAX_PHASE", "E"))
    nfeat, nhid, ncls = cfg["nfeat"], cfg["nhid"], cfg["ncls"]
    ntab, nwin, wrow = cfg["ntab"], cfg["nwin"], cfg["wrow"]
    nloc = cfg["nloc"]
    S = cfg["S"]
    G = nloc // P
    KC = nfeat // P      # k-chunks for transform
    HC = nhid // P       # k-chunks for layer-2 transform
    NHPAD = P            # padded g2 row length (bf16 -> 256B)

    chunks = _chunks(G)
    # per-g block counts and index-column counts
    Bg = [sum(S[g][q] // P for q in range(nwin)) for g in range(G)]
    Cg = [sum(S[g][q] // 16 for q in range(nwin)) for g in range(G)]

    nc = bacc.Bacc(num_swdge_queues=4)
    qrr = [0]  # round-robin SWDGE queue assignment for gathers

    def next_q():
        qrr[0] = (qrr[0] + 1) % 4
        return qrr[0]

    # ---- external inputs ----
    A_in = nc.dram_tensor("A", [G, P, KC * P], BF16, kind="ExternalInput")
    W1_in = nc.dram_tensor("W1c", [P, KC * nhid], BF16, kind="ExternalInput")
    W2_in = nc.dram_tensor("W2c", [P, HC * ncls], BF16, kind="ExternalInput")
    b1r_in = nc.dram_tensor("b1r", [1, nhid], BF16, kind="ExternalInput")
    b2_in = nc.dram_tensor("b2b", [P, ncls], F32, kind="ExternalInput")
    iota_in = nc.dram_tensor("iota", [P, P], F32, kind="ExternalInput")
    degl_in = nc.dram_tensor("degl", [P, G], F32, kind="ExternalInput")
    sdeg_in = nc.dram_tensor("sdegr", [1, nloc], BF16, kind="ExternalInput")
    idx_in = nc.dram_tensor("idx", [P, sum(Cg)], I16, kind="ExternalInput")
    da_in = nc.dram_tensor("da", [P, sum(Bg)], F32, kind="ExternalInput")

    out_ext = nc.dram_tensor("out", [nloc, ncls], F32, kind="ExternalOutput")

    # ---- internal DRAM ----
    dbg = os.environ.get("GCN_DEBUG") == "1"
    g1_loc = nc.dram_tensor("g1_loc", [nloc, nhid], BF16)
    g1_tab = nc.dram_tensor("g1_tab", [ntab, nhid], BF16, addr_space="Shared")
    g2_loc = nc.dram_tensor("g2_loc", [nloc, NHPAD], BF16)
    if dbg:
        g1_dbg = nc.dram_tensor("g1_dbg", [nloc, nhid], BF16,
                                kind="ExternalOutput")
        g2_dbg = nc.dram_tensor("g2_dbg", [nloc, NHPAD], BF16,
                                kind="ExternalOutput")
    g2_tab = nc.dram_tensor("g2_tab", [ntab, NHPAD], BF16, addr_space="Shared")

    AF = mybir.ActivationFunctionType

    with tile.TileContext(nc) as tc:
        with (
            tc.tile_pool(name="const", bufs=1) as cpool,
            tc.tile_pool(name="xf", bufs=3) as xfpool,
            tc.tile_pool(name="meta", bufs=3) as mpool,
            tc.tile_pool(name="gat", bufs=2) as gpool,
            tc.tile_pool(name="sel", bufs=2) as spool,
            tc.tile_pool(name="epi", bufs=3) as epool,
            tc.tile_pool(name="psA", bufs=3, space="PSUM") as psA,
            tc.tile_pool(name="psB", bufs=2, space="PSUM") as psB,
        ):
            # ---- constants ----
            w1_t = cpool.tile([P, KC * nhid], BF16)
            nc.sync.dma_start(out=w1_t[:], in_=W1_in[:, :])
            w2_t = cpool.tile([P, HC * ncls], BF16)
            nc.sync.dma_start(out=w2_t[:], in_=W2_in[:, :])
            b1r_t = cpool.tile([1, nhid], BF16)
            nc.sync.dma_start(out=b1r_t[:], in_=b1r_in[:, :])
            b2_t = cpool.tile([P, ncls], F32)
            nc.sync.dma_start(out=b2_t[:], in_=b2_in[:, :])
            iota_t = cpool.tile([P, P], F32)
            nc.sync.dma_start(out=iota_t[:], in_=iota_in[:, :])
            sdeg_t = cpool.tile([1, nloc], BF16)
            nc.sync.dma_start(out=sdeg_t[:], in_=sdeg_in[:, :])

            degl_t = cpool.tile([P, G], F32)
            nc.sync.dma_start(out=degl_t[:], in_=degl_in[:, :])
            dinvl_t = cpool.tile([P, G], F32)
            nc.vector.reciprocal(out=dinvl_t[:], in_=degl_t[:])
            nc.scalar.activation(out=dinvl_t[:], in_=dinvl_t[:], func=AF.Sqrt)
            dinv2_t = cpool.tile([P, G], F32)
            nc.vector.reciprocal(out=dinv2_t[:], in_=degl_t[:])

            # ---- phase B: sharded transform -> g1_loc ----
            for t in range(G):
                a_t = xfpool.tile([P, KC * P], BF16, tag="a")
                nc.sync.dma_start(out=a_t[:], in_=A_in[t])
                ps = psB.tile([P, nhid], F32, tag="g2")
                for c in range(KC):
                    nc.tensor.matmul(
                        out=ps[:], lhsT=a_t[:, c * P:(c + 1) * P],
                        rhs=w1_t[:, c * nhid:(c + 1) * nhid],
                        start=(c == 0), stop=(c == KC - 1),
                    )
                gsb = xfpool.tile([P, nhid], BF16, tag="gout")
                nc.scalar.activation(out=gsb[:], in_=ps[:], func=AF.Copy,
                                     scale=dinvl_t[:, t:t + 1])
                nc.sync.dma_start(out=g1_loc[t * P:(t + 1) * P, :], in_=gsb[:])
                if dbg:
                    nc.sync.dma_start(out=g1_dbg[t * P:(t + 1) * P, :],
                                      in_=gsb[:])

            # ---- phase G: exchange g1 ----
            if max_phase >= 1:
                nc.gpsimd.collective_compute(
                    "AllGather", mybir.AluOpType.bypass,
                    replica_groups=[list(range(NCORES))],
                    ins=[g1_loc[:, :]], outs=[g1_tab[:, :]],
                )

            # ---- shared chunk machinery for phases C and E ----
            # slot order within a chunk: [q][g in chunk][slots (128-padded)]
            def agg_phase(tab, width, rwidth, emit_accs, emit_epilogue):
                """tab: gather source; width: gathered row elems; rwidth:
                rhs width used by matmuls; emit_accs(chunk)->accs;
                emit_epilogue(g, acc) emits the per-group epilogue."""
                co_base = 0
                bo_base = 0
                pend = []
                for chunk in chunks:
                    ccols = sum(Cg[g] for g in chunk)
                    bcols = sum(Bg[g] for g in chunk)
                    idx_t = mpool.tile([P, ccols], I16, tag="idx")
                    nc.sync.dma_start(out=idx_t[:],
                                      in_=idx_in[:, co_base:co_base + ccols])
                    da_t = mpool.tile([P, bcols], F32, tag="da")
                    nc.sync.dma_start(out=da_t[:],
                                      in_=da_in[:, bo_base:bo_base + bcols])

                    gat_t = gpool.tile([P, bcols, width], BF16, tag="gat")
                    co = 0
                    bo = 0
                    # block layout bookkeeping: (q, g) -> block offset
                    blk_of = {}
                    for q in range(nwin):
                        cell = sum(S[g][q] for g in chunk)
                        b0 = bo
                        for g in chunk:
                            blk_of[(q, g)] = bo
                            bo += S[g][q] // P
                        for s0 in range(0, cell, 1024):
                            ss = min(1024, cell - s0)
                            nc.gpsimd.dma_gather(
                                gat_t[:, b0 + s0 // P:b0 + (s0 + ss) // P, :],
                                tab[q * wrow:(q + 1) * wrow, :],
                                idx_t[:, co + s0 // 16:co + (s0 + ss) // 16],
                                ss, ss, width, queue_num=next_q(),
                            )
                        co += cell // 16

                    sel = spool.tile([P, bcols, P], BF16, tag="sel")
                    nc.vector.tensor_tensor(
                        out=sel[:],
                        in0=da_t[:, 0:bcols].unsqueeze(2)
                            .to_broadcast([P, bcols, P]),
                        in1=iota_t[:].unsqueeze(1).to_broadcast([P, bcols, P]),
                        op=mybir.AluOpType.is_equal)

                    # per-g accumulation (g-major matmul order); epilogue
                    # of the previous group is emitted after the next
                    # group's matmuls (one-step software pipeline)
                    nz = {g: [q for q in range(nwin) if S[g][q] > 0]
                          for g in chunk}
                    for g in chunk:
                        acc = emit_accs(g)
                        first_q, last_q = nz[g][0], nz[g][-1]
                        for q in nz[g]:
                            nb = S[g][q] // P
                            for b in range(nb):
                                bi = blk_of[(q, g)] + b
                                is_first = (q == first_q and b == 0)
                                is_last = (q == last_q and b == nb - 1)
                                emit_accs.mm(acc, gat_t, sel, bi,
                                             is_first, is_last)
                        for (gp, accp) in pend:
                            emit_epilogue(gp, accp)
                        pend = [(g, acc)]

                    co_base += ccols
                    bo_base += bcols
                for (gp, accp) in pend:
                    emit_epilogue(gp, accp)

            # ---- phase C: L1 aggregation + fused layer-2 transform ----
            # accT[hid, d] accumulated transposed; halves of one PSUM tile
            # hold the two hid chunks. out1 = dinv*relu(agg + sqrtdeg*b1);
            # g2 = dinv^2 * (relu(...) @ W2).
            if max_phase >= 2:
                def c_accs(g):
                    acc = [psA.tile([P, P], F32, tag=f"acch{h}",
                                    name=f"acch{h}") for h in range(HC)]
                    for h in range(HC):
                        nc.tensor.matmul(
                            out=acc[h][:],
                            lhsT=b1r_t[0:1, h * P:(h + 1) * P],
                            rhs=sdeg_t[0:1, g * P:(g + 1) * P],
                            start=True, stop=False)
                    return acc

                def c_mm(acc, gat_t, sel, bi, is_first, is_last):
                    for h in range(HC):
                        nc.tensor.matmul(
                            out=acc[h][:],
                            lhsT=gat_t[:, bi, h * P:(h + 1) * P],
                            rhs=sel[:, bi, :],
                            start=False, stop=is_last)
                c_accs.mm = c_mm

                def c_epilogue(g, acc):
                    us = []
                    for h in range(HC):
                        u = epool.tile([P, P], BF16, tag=f"u{h}",
                                       name=f"u{h}")
                        nc.scalar.activation(out=u[:], in_=acc[h][:],
                                             func=AF.Relu)
                        us.append(u)
                    g2ps = psB.tile([P, ncls], F32, tag="g2")
                    for h in range(HC):
                        nc.tensor.matmul(out=g2ps[:], lhsT=us[h][:],
                                         rhs=w2_t[:, h * ncls:(h + 1) * ncls],
                                         start=(h == 0), stop=(h == HC - 1))
                    g2sb = epool.tile([P, NHPAD], BF16, tag="g2sb")
                    nc.scalar.activation(out=g2sb[:, 0:ncls], in_=g2ps[:],
                                         func=AF.Copy,
                                         scale=dinv2_t[:, g:g + 1])
                    nc.sync.dma_start(out=g2_loc[g * P:(g + 1) * P, :],
                                      in_=g2sb[:])
                    if dbg:
                        nc.sync.dma_start(out=g2_dbg[g * P:(g + 1) * P, :],
                                          in_=g2sb[:])

                agg_phase(g1_tab, nhid, nhid, c_accs, c_epilogue)

            # ---- phase D: exchange g2 ----
            if max_phase >= 3:
                nc.gpsimd.collective_compute(
                    "AllGather", mybir.AluOpType.bypass,
                    replica_groups=[list(range(NCORES))],
                    ins=[g2_loc[:, :]], outs=[g2_tab[:, :]],
                )

            # ---- phase E: L2 aggregation + log_softmax ----
            if max_phase >= 4:
                def e_accs(g):
                    acc = psA.tile([P, ncls], F32, tag="acch0",
                                   name="acce")
                    return acc

                def e_mm(acc, gat_t, sel, bi, is_first, is_last):
                    nc.tensor.matmul(out=acc[:], lhsT=sel[:, bi, :],
                                     rhs=gat_t[:, bi, 0:ncls],
                                     start=is_first, stop=is_last)
                e_accs.mm = e_mm

                def e_epilogue(g, acc):
                    t1 = epool.tile([P, ncls], F32, tag="e1")
                    nc.scalar.activation(out=t1[:], in_=acc[:], func=AF.Copy,
                                         scale=dinvl_t[:, g:g + 1])
                    o2 = epool.tile([P, ncls], F32, tag="e2")
                    nc.vector.tensor_tensor(out=o2[:], in0=t1[:], in1=b2_t[:],
                                            op=mybir.AluOpType.add)
                    negm = epool.tile([P, 1], F32, tag="negm")
                    nc.vector.tensor_reduce(out=negm[:], in_=o2[:],
                                            op=mybir.AluOpType.max,
                                            axis=mybir.AxisListType.X,
                                            negate=True)
                    e_t = epool.tile([P, ncls], F32, tag="escr")
                    s_t = epool.tile([P, 1], F32, tag="ssum")
                    nc.scalar.activation(out=e_t[:], in_=o2[:], func=AF.Exp,
                                         bias=negm[:, 0:1],
                                         accum_out=s_t[:, 0:1])
                    l_t = epool.tile([P, 1], F32, tag="lsum")
                    nc.scalar.activation(out=l_t[:], in_=s_t[:], func=AF.Ln)
                    mpl = epool.tile([P, 1], F32, tag="mpl")
                    nc.vector.tensor_tensor(out=mpl[:], in0=l_t[:], in1=negm[:],
                                            op=mybir.AluOpType.subtract)
                    fin = epool.tile([P, ncls], F32, tag="fin")
                    nc.vector.tensor_scalar(out=fin[:], in0=o2[:],
                                            scalar1=mpl[:, 0:1],
                                            scalar2=None,
                                            op0=mybir.AluOpType.subtract)
                    nc.sync.dma_start(out=out_ext[g * P:(g + 1) * P, :],
                                      in_=fin[:])

                agg_phase(g2_tab, NHPAD, ncls, e_accs, e_epilogue)

    nc.compile()
    return nc


# --------------------------------------------------------------------------
# host-side data prep
# --------------------------------------------------------------------------
def _wrap_idx_cols(vals, S):
    """vals: int array of S slot indices -> [128, S//16] int16 (16-wrapped, x8)"""
    w = vals.reshape(S // 16, 16).T.astype(np.int16)  # [16, S/16]
    return np.tile(w, (8, 1))


def prepare(x, edge_index, W1, b1, W2, b2):
    n, nfeat = x.shape
    nhid = W1.shape[1]
    ncls = W2.shape[1]
    assert n % NCORES == 0
    nown = n // NCORES                       # real nodes per core
    nloc = -(-nown // P) * P                 # padded local nodes
    ntab = NCORES * nloc                     # core-major table rows
    nwin = 4
    assert ntab % nwin == 0
    wrow = ntab // nwin
    assert wrow < 32768
    G = nloc // P

    src = np.asarray(edge_index[0], dtype=np.int64)
    dst = np.asarray(edge_index[1], dtype=np.int64)

    deg = np.bincount(dst, minlength=n).astype(np.float32) + 1.0

    # append self loops, sort by dst (stable keeps determinism)
    loops = np.arange(n, dtype=np.int64)
    src_all = np.concatenate([src, loops])
    dst_all = np.concatenate([dst, loops])
    order = np.argsort(dst_all, kind="stable")
    ssrc = src_all[order]
    sdst = dst_all[order]

    # src row in the core-major padded table (shared by both layers)
    core_of = ssrc // nown
    row2 = core_of * nloc + (ssrc - core_of * nown)
    we = row2 // wrow
    ie = (row2 - we * wrow).astype(np.int64)

    # per-core edge ranges (dst owner)
    cuts = np.searchsorted(sdst, np.arange(NCORES + 1) * nown)

    # first pass: per (core, g, q) counts
    cnt = np.zeros((NCORES, G, nwin), np.int64)
    per_core = []
    for k in range(NCORES):
        e0, e1 = cuts[k], cuts[k + 1]
        dl = (sdst[e0:e1] - k * nown).astype(np.int64)
        gid = dl // P
        gcuts = np.searchsorted(gid, np.arange(G + 1))
        per_core.append((e0, e1, dl, gcuts))
        for g in range(G):
            a, b = gcuts[g], gcuts[g + 1]
            cnt[k, g] = np.bincount(we[e0 + a:e0 + b], minlength=nwin)

    m = cnt.max(axis=0)                      # [G, nwin]
    S = (-(-m // P) * P).astype(np.int64)    # pad to 128, 0 stays 0

    chunks = _chunks(G)

    # second pass: build idx/dstadj arrays per core, chunk-major slot order:
    # [chunk][q][g in chunk][slots padded to S[g][q]]
    def build_layer(k):
        e0, e1, dl, gcuts = per_core[k]
        idx_cols = []
        da_cols = []
        for chunk in chunks:
            for q in range(nwin):
                for g in chunk:
                    a, b = gcuts[g], gcuts[g + 1]
                    wv = we[e0 + a:e0 + b]
                    iv = ie[e0 + a:e0 + b]
                    dv = dl[a:b] - g * P
                    S_gq = int(S[g, q])
                    if S_gq == 0:
                        continue
                    msk = wv == q
                    cntq = int(msk.sum())
                    vals = np.zeros(S_gq, np.int64)
                    vals[:cntq] = iv[msk]
                    dd = np.full(S_gq, -1e9, np.float32)
                    dd[:cntq] = dv[msk].astype(np.float32)
                    idx_cols.append(_wrap_idx_cols(vals, S_gq))
                    da_cols.append(dd.reshape(S_gq // P, P).T)
        return (np.concatenate(idx_cols, axis=1),
                np.ascontiguousarray(np.concatenate(da_cols, axis=1)))

    KC = nfeat // P
    HC = nhid // P

    W1c = (np.asarray(W1, np.float32).reshape(KC, P, nhid).transpose(1, 0, 2)
           .reshape(P, KC * nhid).astype(ml_dtypes.bfloat16))
    W2c = (np.asarray(W2, np.float32).reshape(HC, P, ncls).transpose(1, 0, 2)
           .reshape(P, HC * ncls).astype(ml_dtypes.bfloat16))
    b1r = np.asarray(b1, np.float32).reshape(1, nhid).astype(ml_dtypes.bfloat16)
    b2b = np.tile(np.asarray(b2, np.float32), (P, 1))
    iota = np.broadcast_to(np.arange(P, dtype=np.float32), (P, P)).copy()

    in_maps = []
    for k in range(NCORES):
        # per-core transform input: A[t][p, c*P+j] = x[k*nown + t*P+j, c*P+p]
        xk = np.zeros((nloc, nfeat), np.float32)
        xk[:nown] = x[k * nown:(k + 1) * nown]
        A = (xk.T.reshape(KC, P, G, P).transpose(2, 1, 0, 3)
             .reshape(G, P, KC * P).astype(ml_dtypes.bfloat16))

        dloc = np.ones(nloc, np.float32)
        dloc[:nown] = deg[k * nown:(k + 1) * nown]
        degl = dloc.reshape(G, P).T.copy()
        sdegr = np.sqrt(dloc).reshape(1, nloc).astype(ml_dtypes.bfloat16)
        idx, da = build_layer(k)
        in_maps.append({
            "A": A, "W1c": W1c, "W2c": W2c, "b1r": b1r, "b2b": b2b,
            "iota": iota, "degl": degl, "sdegr": sdegr,
            "idx": idx, "da": da,
        })

    cfg = {
        "nfeat": nfeat, "nhid": nhid, "ncls": ncls,
        "ntab": ntab, "nwin": nwin, "wrow": wrow, "nloc": nloc,
        "S": S.tolist(),
    }
    return cfg, in_maps, nown


def _run(x, edge_index, W1, b1, W2, b2, trace=False):
    cfg, in_maps, nown = prepare(x, edge_index, W1, b1, W2, b2)
    key = repr(sorted(cfg.items()))
    nc = _prog_cache.get(key)
    if nc is None:
        nc = build_program(cfg)
        _prog_cache[key] = nc
    res = run_bass_kernel_spmd(nc, in_maps, core_ids=list(range(NCORES)),
                               trace=trace)
    n = x.shape[0]
    ncls = W2.shape[1]
    out = np.empty((n, ncls), np.float32)
    for k in range(NCORES):
        out[k * nown:(k + 1) * nown] = res.results[k]["out"][:nown]
    return out, res


def kernel(x, edge_index, W1, b1, W2, b2):
    out, _ = _run(np.asarray(x), np.asarray(edge_index),
                  np.asarray(W1), np.asarray(b1), np.asarray(W2), np.asarray(b2))
    return out


# --------------------------------------------------------------------------
# timing harness (test.py only): stage inputs once, time repeated executions
# --------------------------------------------------------------------------
def build_timed_runner(nc, in_maps):
    """Mirror run_bass_via_pjrt's multi-core path, but keep inputs staged on
    device and return a callable that executes once and blocks."""
    import jax
    from jax.sharding import Mesh, PartitionSpec
    from jax.experimental.shard_map import shard_map
    from concourse import bass2jax
    from concourse.bass2jax import (_bass_exec_p, partition_id_tensor,
                                    fast_dispatch_compile)

    bass2jax.install_neuronx_cc_hook()
    n_cores = len(in_maps)

    partition_name = nc.partition_id_tensor.name if nc.partition_id_tensor else None
    in_names, out_names, out_avals, zero_outs = [], [], [], []
    for alloc in nc.m.functions[0].allocations:
        if not isinstance(alloc, mybir.MemoryLocationSet):
            continue
        name = alloc.memorylocations[0].name
        if alloc.kind == "ExternalInput":
            if name != partition_name:
                in_names.append(name)
        elif alloc.kind == "ExternalOutput":
            out_names.append(name)
            shape = tuple(alloc.tensor_shape)
            dtype = mybir.dt.np(alloc.dtype)
            out_avals.append(jax.core.ShapedArray(shape, dtype))
            zero_outs.append(np.zeros(shape, dtype))
    n_params = len(in_names)
    all_in_names = in_names + out_names + ([partition_name] if partition_name else [])

    def _body(*args):
        operands = list(args)
        if partition_name is not None:
            operands.append(partition_id_tensor())
        return tuple(_bass_exec_p.bind(
            *operands, out_avals=tuple(out_avals), in_names=tuple(all_in_names),
            out_names=tuple(out_names), lowering_input_output_aliases=(),
            sim_require_finite=True, sim_require_nnan=True, nc=nc))

    devices = jax.devices()[:n_cores]
    mesh = Mesh(np.asarray(devices), ("core",))
    n_outs = len(out_names)

    import time
    t0 = time.time()
    abstract = [jax.ShapeDtypeStruct(
        (n_cores * np.asarray(in_maps[0][nm]).shape[0],
         *np.asarray(in_maps[0][nm]).shape[1:]),
        np.asarray(in_maps[0][nm]).dtype) for nm in in_names]
    abstract += [jax.ShapeDtypeStruct((n_cores * z.shape[0], *z.shape[1:]), z.dtype)
                 for z in zero_outs]

    # No donation: the NEFF binds outputs to fresh result buffers and never
    # reads the zero "output" params, so one staged device-resident zeros
    # array can be reused every call (avoids a 16MB host->device upload
    # through the axon tunnel per execution). fast_dispatch_compile drops
    # the bass effect so dispatch takes the C++ fast path.
    def _compile():
        return jax.jit(
            shard_map(_body, mesh=mesh,
                      in_specs=(PartitionSpec("core"),) * (n_params + n_outs),
                      out_specs=(PartitionSpec("core"),) * n_outs,
                      check_rep=False),
            keep_unused=True).lower(*abstract).compile()

    sharded = fast_dispatch_compile(_compile)
    print(f"[runner] jit+neff compile: {time.time() - t0:.1f}s", flush=True)

    from jax.sharding import NamedSharding
    shard = NamedSharding(mesh, PartitionSpec("core"))
    staged = []
    for i, name in enumerate(in_names):
        cat = np.concatenate([np.asarray(m[name]) for m in in_maps], axis=0)
        staged.append(jax.device_put(cat, shard))
    jax.block_until_ready(staged)
    print(f"[runner] inputs staged: {time.time() - t0:.1f}s", flush=True)

    staged_zeros = []
    for z in zero_outs:
        zz = np.zeros((n_cores * z.shape[0], *z.shape[1:]), z.dtype)
        staged_zeros.append(jax.device_put(zz, shard))
    jax.block_until_ready(staged_zeros)

    def submit():
        return sharded(*staged, *staged_zeros)

    def run_once():
        out = submit()
        jax.block_until_ready(out)
        return out

    run_once.submit = submit
    return run_once, out_names, out_avals


# revision 38
# speedup vs baseline: 1.2180x; 1.1975x over previous
"""Distributed 2-layer GCN (PyG GCNConv semantics) on 8 Trainium2 NeuronCores.

Strategy (graph/data parallel, per sharding hint):
- Nodes sharded by contiguous range across 8 cores; edges sharded by dst owner.
- Node tables (g1 = dinv*(x@W1), g2 = dinv^2*(relu(agg+sqrtdeg*b1)@W2)) use one
  core-major padded row layout, so both layers share a single edge-index
  structure.
- Layer tables are built SHARDED (each core transforms its own nodes) and
  exchanged with AllGather into a Shared DRAM table per core.
- Edge aggregation = sorted-by-dst gather (dma_gather on 4 SWDGE queues,
  1024-index single-packet calls, merged cells over CHUNK dst-groups per
  window) + one-hot selection matmul in PSUM.
- Layer-1 accumulates TRANSPOSED (hid on partitions) so relu applies in place
  and the layer-2 transform needs no PE transpose; b1 enters as a rank-1
  matmul against sqrt(deg).
"""
import numpy as np
import ml_dtypes

import concourse.bass as bass
import concourse.mybir as mybir
import concourse.tile as tile
from concourse import bacc
from concourse.bass_utils import run_bass_kernel_spmd

F32 = mybir.dt.float32
BF16 = mybir.dt.bfloat16
I16 = mybir.dt.int16

P = 128
NCORES = 8
CHUNK = 2          # dst-groups merged per gather cell

# problem sizes (hardcoded per spec)
N_NODES = 100000
NFEAT = 512
NHID = 256
NCLS = 40

_prog_cache = {}

_PHASES = ["B", "G", "C", "D", "E"]


def _chunks(G):
    return [list(range(c, min(c + CHUNK, G))) for c in range(0, G, CHUNK)]


# --------------------------------------------------------------------------
# program builder
# --------------------------------------------------------------------------
def build_program(cfg):
    import os
    max_phase = _PHASES.index(os.environ.get("GCN_MAX_PHASE", "E"))
    nfeat, nhid, ncls = cfg["nfeat"], cfg["nhid"], cfg["ncls"]
    ntab, nwin, wrow = cfg["ntab"], cfg["nwin"], cfg["wrow"]
    nloc = cfg["nloc"]
    S = cfg["S"]
    G = nloc // P
    KC = nfeat // P      # k-chunks for transform
    HC = nhid // P       # k-chunks for layer-2 transform
    NHPAD = P            # padded g2 row length (bf16 -> 256B)

    chunks = _chunks(G)
    # per-g block counts and index-column counts
    Bg = [sum(S[g][q] // P for q in range(nwin)) for g in range(G)]
    Cg = [sum(S[g][q] // 16 for q in range(nwin)) for g in range(G)]

    nc = bacc.Bacc(num_swdge_queues=4)
    qrr = [0]  # round-robin SWDGE queue assignment for gathers

    def next_q():
        qrr[0] = (qrr[0] + 1) % 4
        return qrr[0]

    # ---- external inputs ----
    A_in = nc.dram_tensor("A", [G, P, KC * P], BF16, kind="ExternalInput")
    W1_in = nc.dram_tensor("W1c", [P, KC * nhid], BF16, kind="ExternalInput")
    W2_in = nc.dram_tensor("W2c", [P, HC * ncls], BF16, kind="ExternalInput")
    b1r_in = nc.dram_tensor("b1r", [1, nhid], BF16, kind="ExternalInput")
    b2_in = nc.dram_tensor("b2b", [P, ncls], F32, kind="ExternalInput")
    iota_in = nc.dram_tensor("iota", [P, P], BF16, kind="ExternalInput")
    degl_in = nc.dram_tensor("degl", [P, G], F32, kind="ExternalInput")
    sdeg_in = nc.dram_tensor("sdegr", [1, nloc], BF16, kind="ExternalInput")
    idx_in = nc.dram_tensor("idx", [P, sum(Cg)], I16, kind="ExternalInput")
    da_in = nc.dram_tensor("da", [P, sum(Bg)], BF16, kind="ExternalInput")

    out_ext = nc.dram_tensor("out", [nloc, ncls], F32, kind="ExternalOutput")

    # ---- internal DRAM ----
    dbg = os.environ.get("GCN_DEBUG") == "1"
    g1_loc = nc.dram_tensor("g1_loc", [nloc, nhid], BF16)
    g1_tab = nc.dram_tensor("g1_tab", [ntab, nhid], BF16, addr_space="Shared")
    g2_loc = nc.dram_tensor("g2_loc", [nloc, NHPAD], BF16)
    if dbg:
        g1_dbg = nc.dram_tensor("g1_dbg", [nloc, nhid], BF16,
                                kind="ExternalOutput")
        g2_dbg = nc.dram_tensor("g2_dbg", [nloc, NHPAD], BF16,
                                kind="ExternalOutput")
    g2_tab = nc.dram_tensor("g2_tab", [ntab, NHPAD], BF16, addr_space="Shared")

    AF = mybir.ActivationFunctionType

    with tile.TileContext(nc) as tc:
        with (
            tc.tile_pool(name="const", bufs=1) as cpool,
            tc.tile_pool(name="xf", bufs=3) as xfpool,
            tc.tile_pool(name="meta", bufs=3) as mpool,
            tc.tile_pool(name="gat", bufs=2) as gpool,
            tc.tile_pool(name="sel", bufs=2) as spool,
            tc.tile_pool(name="epi", bufs=3) as epool,
            tc.tile_pool(name="psA", bufs=3, space="PSUM") as psA,
            tc.tile_pool(name="psB", bufs=2, space="PSUM") as psB,
        ):
            # ---- constants ----
            w1_t = cpool.tile([P, KC * nhid], BF16)
            nc.sync.dma_start(out=w1_t[:], in_=W1_in[:, :])
            w2_t = cpool.tile([P, HC * ncls], BF16)
            nc.sync.dma_start(out=w2_t[:], in_=W2_in[:, :])
            b1r_t = cpool.tile([1, nhid], BF16)
            nc.sync.dma_start(out=b1r_t[:], in_=b1r_in[:, :])
            b2_t = cpool.tile([P, ncls], F32)
            nc.sync.dma_start(out=b2_t[:], in_=b2_in[:, :])
            iota_t = cpool.tile([P, P], BF16)
            nc.sync.dma_start(out=iota_t[:], in_=iota_in[:, :])
            sdeg_t = cpool.tile([1, nloc], BF16)
            nc.sync.dma_start(out=sdeg_t[:], in_=sdeg_in[:, :])

            degl_t = cpool.tile([P, G], F32)
            nc.sync.dma_start(out=degl_t[:], in_=degl_in[:, :])
            dinvl_t = cpool.tile([P, G], F32)
            nc.vector.reciprocal(out=dinvl_t[:], in_=degl_t[:])
            nc.scalar.activation(out=dinvl_t[:], in_=dinvl_t[:], func=AF.Sqrt)
            dinv2_t = cpool.tile([P, G], F32)
            nc.vector.reciprocal(out=dinv2_t[:], in_=degl_t[:])

            # ---- phase B: sharded transform -> g1_loc ----
            for t in range(G):
                a_t = xfpool.tile([P, KC * P], BF16, tag="a")
                nc.sync.dma_start(out=a_t[:], in_=A_in[t])
                ps = psB.tile([P, nhid], F32, tag="g2")
                for c in range(KC):
                    nc.tensor.matmul(
                        out=ps[:], lhsT=a_t[:, c * P:(c + 1) * P],
                        rhs=w1_t[:, c * nhid:(c + 1) * nhid],
                        start=(c == 0), stop=(c == KC - 1),
                    )
                gsb = xfpool.tile([P, nhid], BF16, tag="gout")
                nc.scalar.activation(out=gsb[:], in_=ps[:], func=AF.Copy,
                                     scale=dinvl_t[:, t:t + 1])
                nc.sync.dma_start(out=g1_loc[t * P:(t + 1) * P, :], in_=gsb[:])
                if dbg:
                    nc.sync.dma_start(out=g1_dbg[t * P:(t + 1) * P, :],
                                      in_=gsb[:])

            # ---- phase G: exchange g1 ----
            if max_phase >= 1:
                nc.gpsimd.collective_compute(
                    "AllGather", mybir.AluOpType.bypass,
                    replica_groups=[list(range(NCORES))],
                    ins=[g1_loc[:, :]], outs=[g1_tab[:, :]],
                )

            # ---- shared chunk machinery for phases C and E ----
            # slot order within a chunk: [q][g in chunk][slots (128-padded)]
            def agg_phase(tab, width, rwidth, emit_accs, emit_epilogue):
                """tab: gather source; width: gathered row elems; rwidth:
                rhs width used by matmuls; emit_accs(chunk)->accs;
                emit_epilogue(g, acc) emits the per-group epilogue."""
                co_base = 0
                bo_base = 0
                pend = []
                for chunk in chunks:
                    ccols = sum(Cg[g] for g in chunk)
                    bcols = sum(Bg[g] for g in chunk)
                    idx_t = mpool.tile([P, ccols], I16, tag="idx")
                    nc.sync.dma_start(out=idx_t[:],
                                      in_=idx_in[:, co_base:co_base + ccols])
                    da_t = mpool.tile([P, bcols], BF16, tag="da")
                    nc.sync.dma_start(out=da_t[:],
                                      in_=da_in[:, bo_base:bo_base + bcols])

                    gat_t = gpool.tile([P, bcols, width], BF16, tag="gat")
                    co = 0
                    bo = 0
                    # block layout bookkeeping: (q, g) -> block offset
                    blk_of = {}
                    for q in range(nwin):
                        cell = sum(S[g][q] for g in chunk)
                        b0 = bo
                        for g in chunk:
                            blk_of[(q, g)] = bo
                            bo += S[g][q] // P
                        for s0 in range(0, cell, 1024):
                            ss = min(1024, cell - s0)
                            nc.gpsimd.dma_gather(
                                gat_t[:, b0 + s0 // P:b0 + (s0 + ss) // P, :],
                                tab[q * wrow:(q + 1) * wrow, :],
                                idx_t[:, co + s0 // 16:co + (s0 + ss) // 16],
                                ss, ss, width, queue_num=next_q(),
                            )
                        co += cell // 16

                    sel = spool.tile([P, bcols, P], BF16, tag="sel")
                    nc.vector.tensor_tensor(
                        out=sel[:],
                        in0=da_t[:, 0:bcols].unsqueeze(2)
                            .to_broadcast([P, bcols, P]),
                        in1=iota_t[:].unsqueeze(1).to_broadcast([P, bcols, P]),
                        op=mybir.AluOpType.is_equal)

                    # per-g accumulation (g-major matmul order); epilogue
                    # of the previous group is emitted after the next
                    # group's matmuls (one-step software pipeline)
                    nz = {g: [q for q in range(nwin) if S[g][q] > 0]
                          for g in chunk}
                    for g in chunk:
                        acc = emit_accs(g)
                        first_q, last_q = nz[g][0], nz[g][-1]
                        for q in nz[g]:
                            nb = S[g][q] // P
                            for b in range(nb):
                                bi = blk_of[(q, g)] + b
                                is_first = (q == first_q and b == 0)
                                is_last = (q == last_q and b == nb - 1)
                                emit_accs.mm(acc, gat_t, sel, bi,
                                             is_first, is_last)
                        for (gp, accp) in pend:
                            emit_epilogue(gp, accp)
                        pend = [(g, acc)]

                    co_base += ccols
                    bo_base += bcols
                for (gp, accp) in pend:
                    emit_epilogue(gp, accp)

            # ---- phase C: L1 aggregation + fused layer-2 transform ----
            # accT[hid, d] accumulated transposed; halves of one PSUM tile
            # hold the two hid chunks. out1 = dinv*relu(agg + sqrtdeg*b1);
            # g2 = dinv^2 * (relu(...) @ W2).
            if max_phase >= 2:
                def c_accs(g):
                    acc = [psA.tile([P, P], F32, tag=f"acch{h}",
                                    name=f"acch{h}") for h in range(HC)]
                    for h in range(HC):
                        nc.tensor.matmul(
                            out=acc[h][:],
                            lhsT=b1r_t[0:1, h * P:(h + 1) * P],
                            rhs=sdeg_t[0:1, g * P:(g + 1) * P],
                            start=True, stop=False)
                    return acc

                def c_mm(acc, gat_t, sel, bi, is_first, is_last):
                    for h in range(HC):
                        nc.tensor.matmul(
                            out=acc[h][:],
                            lhsT=gat_t[:, bi, h * P:(h + 1) * P],
                            rhs=sel[:, bi, :],
                            start=False, stop=is_last)
                c_accs.mm = c_mm

                def c_epilogue(g, acc):
                    us = []
                    for h in range(HC):
                        u = epool.tile([P, P], BF16, tag=f"u{h}",
                                       name=f"u{h}")
                        nc.scalar.activation(out=u[:], in_=acc[h][:],
                                             func=AF.Relu)
                        us.append(u)
                    g2ps = psB.tile([P, ncls], F32, tag="g2")
                    for h in range(HC):
                        nc.tensor.matmul(out=g2ps[:], lhsT=us[h][:],
                                         rhs=w2_t[:, h * ncls:(h + 1) * ncls],
                                         start=(h == 0), stop=(h == HC - 1))
                    g2sb = epool.tile([P, NHPAD], BF16, tag="g2sb")
                    nc.scalar.activation(out=g2sb[:, 0:ncls], in_=g2ps[:],
                                         func=AF.Copy,
                                         scale=dinv2_t[:, g:g + 1])
                    nc.sync.dma_start(out=g2_loc[g * P:(g + 1) * P, :],
                                      in_=g2sb[:])
                    if dbg:
                        nc.sync.dma_start(out=g2_dbg[g * P:(g + 1) * P, :],
                                          in_=g2sb[:])

                agg_phase(g1_tab, nhid, nhid, c_accs, c_epilogue)

            # ---- phase D: exchange g2 ----
            if max_phase >= 3:
                nc.gpsimd.collective_compute(
                    "AllGather", mybir.AluOpType.bypass,
                    replica_groups=[list(range(NCORES))],
                    ins=[g2_loc[:, :]], outs=[g2_tab[:, :]],
                )

            # ---- phase E: L2 aggregation + log_softmax ----
            if max_phase >= 4:
                def e_accs(g):
                    acc = psA.tile([P, ncls], F32, tag="acch0",
                                   name="acce")
                    return acc

                def e_mm(acc, gat_t, sel, bi, is_first, is_last):
                    nc.tensor.matmul(out=acc[:], lhsT=sel[:, bi, :],
                                     rhs=gat_t[:, bi, 0:ncls],
                                     start=is_first, stop=is_last)
                e_accs.mm = e_mm

                def e_epilogue(g, acc):
                    t1 = epool.tile([P, ncls], F32, tag="e1")
                    nc.scalar.activation(out=t1[:], in_=acc[:], func=AF.Copy,
                                         scale=dinvl_t[:, g:g + 1])
                    o2 = epool.tile([P, ncls], F32, tag="e2")
                    nc.vector.tensor_tensor(out=o2[:], in0=t1[:], in1=b2_t[:],
                                            op=mybir.AluOpType.add)
                    negm = epool.tile([P, 1], F32, tag="negm")
                    nc.vector.tensor_reduce(out=negm[:], in_=o2[:],
                                            op=mybir.AluOpType.max,
                                            axis=mybir.AxisListType.X,
                                            negate=True)
                    e_t = epool.tile([P, ncls], F32, tag="escr")
                    s_t = epool.tile([P, 1], F32, tag="ssum")
                    nc.scalar.activation(out=e_t[:], in_=o2[:], func=AF.Exp,
                                         bias=negm[:, 0:1],
                                         accum_out=s_t[:, 0:1])
                    l_t = epool.tile([P, 1], F32, tag="lsum")
                    nc.scalar.activation(out=l_t[:], in_=s_t[:], func=AF.Ln)
                    mpl = epool.tile([P, 1], F32, tag="mpl")
                    nc.vector.tensor_tensor(out=mpl[:], in0=l_t[:], in1=negm[:],
                                            op=mybir.AluOpType.subtract)
                    fin = epool.tile([P, ncls], F32, tag="fin")
                    nc.vector.tensor_scalar(out=fin[:], in0=o2[:],
                                            scalar1=mpl[:, 0:1],
                                            scalar2=None,
                                            op0=mybir.AluOpType.subtract)
                    nc.sync.dma_start(out=out_ext[g * P:(g + 1) * P, :],
                                      in_=fin[:])

                agg_phase(g2_tab, NHPAD, ncls, e_accs, e_epilogue)

    nc.compile()
    return nc


# --------------------------------------------------------------------------
# host-side data prep
# --------------------------------------------------------------------------
def _wrap_idx_cols(vals, S):
    """vals: int array of S slot indices -> [128, S//16] int16 (16-wrapped, x8)"""
    w = vals.reshape(S // 16, 16).T.astype(np.int16)  # [16, S/16]
    return np.tile(w, (8, 1))


def prepare(x, edge_index, W1, b1, W2, b2):
    n, nfeat = x.shape
    nhid = W1.shape[1]
    ncls = W2.shape[1]
    assert n % NCORES == 0
    nown = n // NCORES                       # real nodes per core
    nloc = -(-nown // P) * P                 # padded local nodes
    ntab = NCORES * nloc                     # core-major table rows
    nwin = 4
    assert ntab % nwin == 0
    wrow = ntab // nwin
    assert wrow < 32768
    G = nloc // P

    src = np.asarray(edge_index[0], dtype=np.int64)
    dst = np.asarray(edge_index[1], dtype=np.int64)

    deg = np.bincount(dst, minlength=n).astype(np.float32) + 1.0

    # append self loops, sort by dst (stable keeps determinism)
    loops = np.arange(n, dtype=np.int64)
    src_all = np.concatenate([src, loops])
    dst_all = np.concatenate([dst, loops])
    order = np.argsort(dst_all, kind="stable")
    ssrc = src_all[order]
    sdst = dst_all[order]

    # src row in the core-major padded table (shared by both layers)
    core_of = ssrc // nown
    row2 = core_of * nloc + (ssrc - core_of * nown)
    we = row2 // wrow
    ie = (row2 - we * wrow).astype(np.int64)

    # per-core edge ranges (dst owner)
    cuts = np.searchsorted(sdst, np.arange(NCORES + 1) * nown)

    # first pass: per (core, g, q) counts
    cnt = np.zeros((NCORES, G, nwin), np.int64)
    per_core = []
    for k in range(NCORES):
        e0, e1 = cuts[k], cuts[k + 1]
        dl = (sdst[e0:e1] - k * nown).astype(np.int64)
        gid = dl // P
        gcuts = np.searchsorted(gid, np.arange(G + 1))
        per_core.append((e0, e1, dl, gcuts))
        for g in range(G):
            a, b = gcuts[g], gcuts[g + 1]
            cnt[k, g] = np.bincount(we[e0 + a:e0 + b], minlength=nwin)

    m = cnt.max(axis=0)                      # [G, nwin]
    S = (-(-m // P) * P).astype(np.int64)    # pad to 128, 0 stays 0

    chunks = _chunks(G)

    # second pass: build idx/dstadj arrays per core, chunk-major slot order:
    # [chunk][q][g in chunk][slots padded to S[g][q]]
    def build_layer(k):
        e0, e1, dl, gcuts = per_core[k]
        idx_cols = []
        da_cols = []
        for chunk in chunks:
            for q in range(nwin):
                for g in chunk:
                    a, b = gcuts[g], gcuts[g + 1]
                    wv = we[e0 + a:e0 + b]
                    iv = ie[e0 + a:e0 + b]
                    dv = dl[a:b] - g * P
                    S_gq = int(S[g, q])
                    if S_gq == 0:
                        continue
                    msk = wv == q
                    cntq = int(msk.sum())
                    vals = np.zeros(S_gq, np.int64)
                    vals[:cntq] = iv[msk]
                    dd = np.full(S_gq, -1e9, np.float32)
                    dd[:cntq] = dv[msk].astype(np.float32)
                    idx_cols.append(_wrap_idx_cols(vals, S_gq))
                    da_cols.append(dd.reshape(S_gq // P, P).T
                                   .astype(ml_dtypes.bfloat16))
        return (np.concatenate(idx_cols, axis=1),
                np.ascontiguousarray(np.concatenate(da_cols, axis=1)))

    KC = nfeat // P
    HC = nhid // P

    W1c = (np.asarray(W1, np.float32).reshape(KC, P, nhid).transpose(1, 0, 2)
           .reshape(P, KC * nhid).astype(ml_dtypes.bfloat16))
    W2c = (np.asarray(W2, np.float32).reshape(HC, P, ncls).transpose(1, 0, 2)
           .reshape(P, HC * ncls).astype(ml_dtypes.bfloat16))
    b1r = np.asarray(b1, np.float32).reshape(1, nhid).astype(ml_dtypes.bfloat16)
    b2b = np.tile(np.asarray(b2, np.float32), (P, 1))
    iota = (np.broadcast_to(np.arange(P, dtype=np.float32), (P, P))
            .astype(ml_dtypes.bfloat16))

    in_maps = []
    for k in range(NCORES):
        # per-core transform input: A[t][p, c*P+j] = x[k*nown + t*P+j, c*P+p]
        xk = np.zeros((nloc, nfeat), np.float32)
        xk[:nown] = x[k * nown:(k + 1) * nown]
        A = (xk.T.reshape(KC, P, G, P).transpose(2, 1, 0, 3)
             .reshape(G, P, KC * P).astype(ml_dtypes.bfloat16))

        dloc = np.ones(nloc, np.float32)
        dloc[:nown] = deg[k * nown:(k + 1) * nown]
        degl = dloc.reshape(G, P).T.copy()
        sdegr = np.sqrt(dloc).reshape(1, nloc).astype(ml_dtypes.bfloat16)
        idx, da = build_layer(k)
        in_maps.append({
            "A": A, "W1c": W1c, "W2c": W2c, "b1r": b1r, "b2b": b2b,
            "iota": iota, "degl": degl, "sdegr": sdegr,
            "idx": idx, "da": da,
        })

    cfg = {
        "nfeat": nfeat, "nhid": nhid, "ncls": ncls,
        "ntab": ntab, "nwin": nwin, "wrow": wrow, "nloc": nloc,
        "S": S.tolist(),
    }
    return cfg, in_maps, nown


def _run(x, edge_index, W1, b1, W2, b2, trace=False):
    cfg, in_maps, nown = prepare(x, edge_index, W1, b1, W2, b2)
    key = repr(sorted(cfg.items()))
    nc = _prog_cache.get(key)
    if nc is None:
        nc = build_program(cfg)
        _prog_cache[key] = nc
    res = run_bass_kernel_spmd(nc, in_maps, core_ids=list(range(NCORES)),
                               trace=trace)
    n = x.shape[0]
    ncls = W2.shape[1]
    out = np.empty((n, ncls), np.float32)
    for k in range(NCORES):
        out[k * nown:(k + 1) * nown] = res.results[k]["out"][:nown]
    return out, res


def kernel(x, edge_index, W1, b1, W2, b2):
    out, _ = _run(np.asarray(x), np.asarray(edge_index),
                  np.asarray(W1), np.asarray(b1), np.asarray(W2), np.asarray(b2))
    return out


# --------------------------------------------------------------------------
# timing harness (test.py only): stage inputs once, time repeated executions
# --------------------------------------------------------------------------
def build_timed_runner(nc, in_maps):
    """Mirror run_bass_via_pjrt's multi-core path, but keep inputs staged on
    device and return a callable that executes once and blocks."""
    import jax
    from jax.sharding import Mesh, PartitionSpec
    from jax.experimental.shard_map import shard_map
    from concourse import bass2jax
    from concourse.bass2jax import (_bass_exec_p, partition_id_tensor,
                                    fast_dispatch_compile)

    bass2jax.install_neuronx_cc_hook()
    n_cores = len(in_maps)

    partition_name = nc.partition_id_tensor.name if nc.partition_id_tensor else None
    in_names, out_names, out_avals, zero_outs = [], [], [], []
    for alloc in nc.m.functions[0].allocations:
        if not isinstance(alloc, mybir.MemoryLocationSet):
            continue
        name = alloc.memorylocations[0].name
        if alloc.kind == "ExternalInput":
            if name != partition_name:
                in_names.append(name)
        elif alloc.kind == "ExternalOutput":
            out_names.append(name)
            shape = tuple(alloc.tensor_shape)
            dtype = mybir.dt.np(alloc.dtype)
            out_avals.append(jax.core.ShapedArray(shape, dtype))
            zero_outs.append(np.zeros(shape, dtype))
    n_params = len(in_names)
    all_in_names = in_names + out_names + ([partition_name] if partition_name else [])

    def _body(*args):
        operands = list(args)
        if partition_name is not None:
            operands.append(partition_id_tensor())
        return tuple(_bass_exec_p.bind(
            *operands, out_avals=tuple(out_avals), in_names=tuple(all_in_names),
            out_names=tuple(out_names), lowering_input_output_aliases=(),
            sim_require_finite=True, sim_require_nnan=True, nc=nc))

    devices = jax.devices()[:n_cores]
    mesh = Mesh(np.asarray(devices), ("core",))
    n_outs = len(out_names)

    import time
    t0 = time.time()
    abstract = [jax.ShapeDtypeStruct(
        (n_cores * np.asarray(in_maps[0][nm]).shape[0],
         *np.asarray(in_maps[0][nm]).shape[1:]),
        np.asarray(in_maps[0][nm]).dtype) for nm in in_names]
    abstract += [jax.ShapeDtypeStruct((n_cores * z.shape[0], *z.shape[1:]), z.dtype)
                 for z in zero_outs]

    # No donation: the NEFF binds outputs to fresh result buffers and never
    # reads the zero "output" params, so one staged device-resident zeros
    # array can be reused every call (avoids a 16MB host->device upload
    # through the axon tunnel per execution). fast_dispatch_compile drops
    # the bass effect so dispatch takes the C++ fast path.
    def _compile():
        return jax.jit(
            shard_map(_body, mesh=mesh,
                      in_specs=(PartitionSpec("core"),) * (n_params + n_outs),
                      out_specs=(PartitionSpec("core"),) * n_outs,
                      check_rep=False),
            keep_unused=True).lower(*abstract).compile()

    sharded = fast_dispatch_compile(_compile)
    print(f"[runner] jit+neff compile: {time.time() - t0:.1f}s", flush=True)

    from jax.sharding import NamedSharding
    shard = NamedSharding(mesh, PartitionSpec("core"))
    staged = []
    for i, name in enumerate(in_names):
        cat = np.concatenate([np.asarray(m[name]) for m in in_maps], axis=0)
        staged.append(jax.device_put(cat, shard))
    jax.block_until_ready(staged)
    print(f"[runner] inputs staged: {time.time() - t0:.1f}s", flush=True)

    staged_zeros = []
    for z in zero_outs:
        zz = np.zeros((n_cores * z.shape[0], *z.shape[1:]), z.dtype)
        staged_zeros.append(jax.device_put(zz, shard))
    jax.block_until_ready(staged_zeros)

    def submit():
        return sharded(*staged, *staged_zeros)

    def run_once():
        out = submit()
        jax.block_until_ready(out)
        return out

    run_once.submit = submit
    return run_once, out_names, out_avals
